# revision 18
# baseline (speedup 1.0000x reference)
"""Bidirectional-LSTM encoder kernel for Trainium2 (8 NeuronCores).

Strategy
--------
Data-parallel over batch: core k owns batch rows [8k, 8k+8) and runs BOTH
LSTM directions as two interleaved serial chains (they hide each other's
engine latency).

Host-side folding (pure linear algebra, done once in numpy):
  * The input projections (leftW/rightW @ concat-embeddings + bias) are
    linear in each embedding table, so they fold into pre-projected tables:
        charTab_d [6000, 400]  = charW @ Wd_char.T + extCharW @ Wd_ext.T + bias_d
        biTab_d [200001, 400]  = bicharW @ Wd_bi.T + extBiCharW @ Wd_extbi.T
    (row 200000 of biTab is zero - used for the left-shift pad position).
    Then  leftIn[b,t] = tanh(charTab_l[cIdx[b,t]] + biTab_l[biIdx[b,t+1]]).
  * LSTM gate rows of Wih/Whh are permuted so the on-chip PSUM layout
    (4 column-tiles x [f|i|o|g] x 100 units) is produced directly.

Device:
  phase 1: indirect-DMA gathers -> add -> tanh -> PE transpose ->
           leftIn^T resident in SBUF -> xW = leftIn^T.T @ WihT (bf16) ->
           packed [T, 32, 400] per direction in DRAM.
  phase 2: per step: scatter-identity matmul preloads xW into PSUM,
           16 col-tiled matmuls accumulate h @ WhhT, ACT sigmoid/tanh,
           DVE gate math, PE transpose of h for the next step.
"""

import numpy as np
import ml_dtypes

import concourse.bacc as bacc
import concourse.bass as bass
import concourse.mybir as mybir
import concourse.tile as tile
from concourse.bass_utils import run_bass_kernel_spmd

BF16 = mybir.dt.bfloat16
F32 = mybir.dt.float32
I32 = mybir.dt.int32

CHAR_NUM = 6000
BICHAR_NUM = 200000
R = 400            # rnn hidden
G = 4 * R          # gate width 1600
B = 64             # full batch
T = 512            # sequence length
BC = 8             # batch rows per core
NCORES = 8

# gate free-order inside each 100-unit quarter: [f | i | o | g]
_GATE_OFF = {0: R, 1: 0, 2: 3 * R, 3: 2 * R}  # f, i, o, g -> row offset in 4R


def _gate_perm():
    """perm[n] = original Wih/Whh row for permuted gate column n."""
    perm = np.empty(G, dtype=np.int64)
    n = 0
    for j in range(4):            # unit quarter
        for s in range(4):        # f, i, o, g
            for u in range(100):
                perm[n] = _GATE_OFF[s] + 100 * j + u
                n += 1
    return perm


def build_nc(t_steps=T):
    """Build the per-core Bass kernel (identical NEFF on all 8 cores)."""
    nc = bacc.Bacc("TRN2", target_bir_lowering=False, debug=False)
    TPB = t_steps // 128                # 128-token tiles per batch row
    NT = BC * TPB                       # number of 128-token gather tiles
    ntok = BC * t_steps

    # ---- external inputs ----
    cidx = nc.dram_tensor("cidx", [128, NT], I32, kind="ExternalInput")
    bidx_l = nc.dram_tensor("bidx_l", [128, NT], I32, kind="ExternalInput")
    bidx_r = nc.dram_tensor("bidx_r", [128, NT], I32, kind="ExternalInput")
    chtab_l = nc.dram_tensor("chtab_l", [CHAR_NUM, R], BF16, kind="ExternalInput")
    chtab_r = nc.dram_tensor("chtab_r", [CHAR_NUM, R], BF16, kind="ExternalInput")
    bitab_l = nc.dram_tensor("bitab_l", [BICHAR_NUM + 1, R], BF16, kind="ExternalInput")
    bitab_r = nc.dram_tensor("bitab_r", [BICHAR_NUM + 1, R], BF16, kind="ExternalInput")
    wih_l = nc.dram_tensor("wih_l", [R + 1, G], BF16, kind="ExternalInput")
    wih_r = nc.dram_tensor("wih_r", [R + 1, G], BF16, kind="ExternalInput")
    whh_l = nc.dram_tensor("whh_l", [R, G], BF16, kind="ExternalInput")
    whh_r = nc.dram_tensor("whh_r", [R, G], BF16, kind="ExternalInput")
    e32 = nc.dram_tensor("e32", [32, 128], BF16, kind="ExternalInput")
    ident = nc.dram_tensor("ident", [128, 128], BF16, kind="ExternalInput")
    ident_f = nc.dram_tensor("ident_f", [128, 128], F32, kind="ExternalInput")

    out_l = nc.dram_tensor("out_l", [t_steps, BC, R], F32, kind="ExternalOutput")
    out_r = nc.dram_tensor("out_r", [t_steps, BC, R], F32, kind="ExternalOutput")

    idx_d = {"l": bidx_l, "r": bidx_r}
    chtab_d = {"l": chtab_l, "r": chtab_r}
    bitab_d = {"l": bitab_l, "r": bitab_r}
    wih_d = {"l": wih_l, "r": wih_r}
    whh_d = {"l": whh_l, "r": whh_r}
    out_d = {"l": out_l, "r": out_r}

    KCH = [(0, 128), (128, 128), (256, 128), (384, 16)]  # leftInT k-chunks

    with tile.TileContext(nc) as tc:
        with (
            tc.tile_pool(name="persist", bufs=1) as pp,
            tc.tile_pool(name="dram", bufs=1, space="DRAM") as dp,
        ):
            # persistent SBUF
            e32_t = pp.tile([32, 128], BF16, tag="e32")
            nc.sync.dma_start(e32_t[:], e32[:])
            id_t = pp.tile([128, 128], BF16, tag="ident")
            nc.sync.dma_start(id_t[:], ident[:])
            idf_t = pp.tile([128, 128], F32, tag="identf")
            nc.sync.dma_start(idf_t[:], ident_f[:])

            idx_t = {}
            for name, src in (("c", cidx), ("l", bidx_l), ("r", bidx_r)):
                t_ = pp.tile([128, NT], I32, tag=f"idx{name}")
                nc.sync.dma_start(t_[:], src[:])
                idx_t[name] = t_

            whh_t = {d: [] for d in "lr"}
            wih_t = {d: [] for d in "lr"}
            for d in "lr":
                for q in range(4):
                    w = pp.tile([100, G], BF16, tag=f"whh{d}{q}")
                    nc.sync.dma_start(w[:], whh_d[d][100 * q:100 * q + 100, :])
                    whh_t[d].append(w)
                for (k0, kw) in KCH:
                    w = pp.tile([kw, G], BF16, tag=f"wih{d}{k0}")
                    nc.sync.dma_start(w[:], wih_d[d][k0:k0 + kw, :])
                    wih_t[d].append(w)
                wb = pp.tile([1, G], BF16, tag=f"wb{d}")
                nc.sync.dma_start(wb[:], wih_d[d][R:R + 1, :])
                wih_t[d].append(wb)

            # leftIn^T resident tiles: 4 k-chunks x [kw, ntok]
            lt = {d: [pp.tile([kw, ntok], BF16, tag=f"lt{d}{k0}", name=f"lt{d}{k0}")
                      for (k0, kw) in KCH] for d in "lr"}
            ones_t = pp.tile([1, ntok], BF16, tag="ones")
            nc.vector.memset(ones_t[:], 1.0)

            # packed xW in DRAM: [t, 8j+b, 400]
            xw = {d: dp.tile([t_steps, 32, R], BF16, tag=f"xw{d}", name=f"xw{d}") for d in "lr"}

            # ---------------- phase 1 ----------------
            with (
                tc.tile_pool(name="p1", bufs=4) as p1,
                tc.tile_pool(name="p1ps", bufs=4, space="PSUM") as p1ps,
                tc.tile_pool(name="p1psx", bufs=4, space="PSUM") as p1psx,
            ):
                for d in "lr":
                    # m-tile order: left wants small t first, right wants big t
                    tt_order = range(TPB) if d == "l" else range(TPB - 1, -1, -1)
                    for tt in tt_order:
                        for b in range(BC):
                            m = b * TPB + tt
                            col0 = 128 * m
                            g1 = p1.tile([128, R], BF16, tag="g1")
                            nc.gpsimd.indirect_dma_start(
                                out=g1[:], out_offset=None, in_=chtab_d[d][:],
                                in_offset=bass.IndirectOffsetOnAxis(
                                    ap=idx_t["c"][:, m:m + 1], axis=0))
                            g2 = p1.tile([128, R], BF16, tag="g2")
                            nc.gpsimd.indirect_dma_start(
                                out=g2[:], out_offset=None, in_=bitab_d[d][:],
                                in_offset=bass.IndirectOffsetOnAxis(
                                    ap=idx_t[d][:, m:m + 1], axis=0))
                            li = p1.tile([128, R], BF16, tag="li")
                            nc.vector.tensor_add(li[:], g1[:], g2[:])
                            nc.scalar.activation(
                                li[:], li[:], mybir.ActivationFunctionType.Tanh)
                            # transpose into leftIn^T chunks
                            for ci, (k0, kw) in enumerate(KCH):
                                w = min(kw, 128) if ci < 3 else 16
                                tps = p1ps.tile([128, 128], BF16, tag="tp",
                                                space="PSUM")
                                nc.tensor.transpose(
                                    out=tps[0:w, :], in_=li[:, k0:k0 + w],
                                    identity=id_t[:])
                                nc.vector.tensor_copy(
                                    lt[d][ci][0:w, col0:col0 + 128], tps[0:w, :])
                        # xW for this tt-group right away (so recurrence can start)
                        for b in range(BC):
                            m = b * TPB + tt
                            col0 = 128 * m
                            for j in range(4):
                                ps = p1psx.tile([128, R], F32, tag="xps",
                                                space="PSUM")
                                nc.tensor.matmul(
                                    ps[:], lhsT=ones_t[:, col0:col0 + 128],
                                    rhs=wih_t[d][4][:, R * j:R * j + R],
                                    start=True, stop=False)
                                for ci, (k0, kw) in enumerate(KCH):
                                    nc.tensor.matmul(
                                        ps[:], lhsT=lt[d][ci][:, col0:col0 + 128],
                                        rhs=wih_t[d][ci][:, R * j:R * j + R],
                                        start=False, stop=(ci == 3))
                                xs = p1.tile([128, R], BF16, tag="xs")
                                nc.scalar.copy(xs[:], ps[:])
                                # dest: xw[t0+dt, 8j+b, :]
                                t0 = 128 * tt
                                nc.sync.dma_start(
                                    xw[d][t0:t0 + 128, 8 * j + b, :], xs[:])

            # ---------------- phase 2 ----------------
            with (
                tc.tile_pool(name="p2xw", bufs=6) as p2xw,
                tc.tile_pool(name="p2", bufs=2) as p2,
                tc.tile_pool(name="p2g", bufs=2, space="PSUM") as p2g,
                tc.tile_pool(name="p2t", bufs=2, space="PSUM") as p2t,
            ):
                # chain state (manually double-buffered)
                cb = {d: [pp.tile([128, 200], F32, tag=f"cb{d}{i}", name=f"cb{d}{i}")
                          for i in range(2)] for d in "lr"}
                ht = {d: [pp.tile([100, 128], BF16, tag=f"ht{d}{i}", name=f"ht{d}{i}")
                          for i in range(2)] for d in "lr"}
                for d in "lr":
                    nc.vector.memset(cb[d][0][:, 0:100], 0.0)
                    nc.vector.memset(ht[d][0][:], 0.0)

                SIG = mybir.ActivationFunctionType.Sigmoid
                TANH = mybir.ActivationFunctionType.Tanh
                MUL = mybir.AluOpType.mult
                ADD = mybir.AluOpType.add

                for s in range(t_steps):
                    for d in "lr":
                        t = s if d == "l" else t_steps - 1 - s
                        prev, cur = cb[d][s % 2], cb[d][(s + 1) % 2]
                        htp, htc = ht[d][s % 2], ht[d][(s + 1) % 2]

                        xwt = p2xw.tile([32, R], BF16, tag=f"xw{d}")
                        nc.sync.dma_start(xwt[:], xw[d][t])

                        ps = p2g.tile([128, R], F32, tag=f"g{d}", space="PSUM")
                        nc.tensor.matmul(ps[:], lhsT=e32_t[:], rhs=xwt[:],
                                         start=True, stop=False)
                        for q in range(4):
                            for j in range(4):
                                nc.tensor.matmul(
                                    ps[32 * j:32 * j + BC, :],
                                    lhsT=htp[:, 32 * q:32 * q + BC],
                                    rhs=whh_t[d][q][:, R * j:R * j + R],
                                    start=False, stop=(q == 3 and j == 3),
                                    tile_position=(0, 32 * j),
                                    skip_group_check=True)

                        sg3 = p2.tile([128, 300], F32, tag=f"sg{d}")
                        nc.scalar.activation(sg3[:], ps[:, 0:300], SIG)
                        nc.scalar.activation(prev[:, 100:200], ps[:, 300:400], TANH)
                        prod = p2.tile([128, 200], F32, tag=f"pr{d}")
                        nc.vector.tensor_tensor(prod[:], sg3[:, 0:200],
                                                prev[:, 0:200], MUL)
                        nc.vector.tensor_tensor(cur[:, 0:100], prod[:, 0:100],
                                                prod[:, 100:200], ADD)
                        thc = p2.tile([128, 100], F32, tag=f"th{d}")
                        nc.scalar.activation(thc[:], cur[:, 0:100], TANH)
                        hb = p2.tile([128, 100], F32, tag=f"hb{d}")
                        nc.vector.tensor_tensor(hb[:], sg3[:, 200:300], thc[:], MUL)

                        # output: out_d[t, b, 100j+u] <- hb[32j+b, u]
                        for j in range(4):
                            nc.sync.dma_start(
                                out_d[d][t, :, 100 * j:100 * j + 100],
                                hb[32 * j:32 * j + BC, :])

                        # h^T for next step (fp32 transpose, cast on copy)
                        tp = p2t.tile([100, 128], F32, tag=f"tp{d}", space="PSUM")
                        nc.tensor.transpose(out=tp[:], in_=hb[:], identity=idf_t[:])
                        nc.vector.tensor_copy(htc[:], tp[:])
    nc.finalize()
    return nc


# ---------------------------------------------------------------- host side

def _prep_weights(inp, t_steps=T):
    """Fold projections into gather tables; permute gate weights. Returns
    dict of np arrays shared by all cores."""
    f32 = np.float32
    leftW = np.asarray(inp["leftW"], f32)
    rightW = np.asarray(inp["rightW"], f32)
    leftb = np.asarray(inp["leftb"], f32)
    rightb = np.asarray(inp["rightb"], f32)
    charW = np.asarray(inp["charW"], f32)
    extCharW = np.asarray(inp["extCharW"], f32)
    bicharW = np.asarray(inp["bicharW"], f32)
    extBiCharW = np.asarray(inp["extBiCharW"], f32)

    # concat order: [char, extChar, biChar', extBiChar']  (see reference)
    def char_tab(Wf, bias):
        return (charW @ Wf[:, 0:200].T + extCharW @ Wf[:, 200:400].T + bias)

    def bi_tab(Wf):
        t = bicharW @ Wf[:, 400:600].T + extBiCharW @ Wf[:, 600:800].T
        return np.concatenate([t, np.zeros((1, R), f32)], axis=0)

    perm = _gate_perm()
    bf16 = ml_dtypes.bfloat16

    def gate_w(wih, whh, bih, bhh):
        wih = np.asarray(wih, f32)[perm, :]          # [G, R]
        whh = np.asarray(whh, f32)[perm, :]
        btot = (np.asarray(bih, f32) + np.asarray(bhh, f32))[perm]
        wih_aug = np.concatenate([wih.T, btot[None, :]], axis=0)  # [R+1, G]
        return wih_aug.astype(bf16), whh.T.astype(bf16)

    wih_l, whh_l = gate_w(inp["Wih_l"], inp["Whh_l"], inp["bih_l"], inp["bhh_l"])
    wih_r, whh_r = gate_w(inp["Wih_r"], inp["Whh_r"], inp["bih_r"], inp["bhh_r"])

    e32 = np.zeros((32, 128), f32)
    for j in range(4):
        for b in range(8):
            e32[8 * j + b, 32 * j + b] = 1.0
    ident = np.eye(128, dtype=f32)

    return {
        "chtab_l": char_tab(leftW, leftb).astype(bf16),
        "chtab_r": char_tab(rightW, rightb).astype(bf16),
        "bitab_l": bi_tab(leftW).astype(bf16),
        "bitab_r": bi_tab(rightW).astype(bf16),
        "wih_l": wih_l, "wih_r": wih_r,
        "whh_l": whh_l, "whh_r": whh_r,
        "e32": e32.astype(bf16), "ident": ident.astype(bf16),
        "ident_f": ident,
    }


def _idx_calls(idx_rows, t_steps=T):
    """[BC, t_steps] int -> [128, NT] gather-call layout."""
    TPB = t_steps // 128
    NT = BC * TPB
    out = np.empty((128, NT), np.int32)
    for m in range(NT):
        b, tt = m // TPB, m % TPB
        out[:, m] = idx_rows[b, 128 * tt:128 * tt + 128]
    return out


_NC_CACHE = {}


def kernel(charIndexes, bicharIndexes, hidden, extCharW, extBiCharW, charW,
           bicharW, leftW, leftb, rightW, rightb, Wih_l, Whh_l, bih_l, bhh_l,
           Wih_r, Whh_r, bih_r, bhh_r, batch=None, _t_steps=None, _trace=False,
           **_unused):
    t_steps = _t_steps or T
    inp = dict(charIndexes=charIndexes, bicharIndexes=bicharIndexes,
               extCharW=extCharW, extBiCharW=extBiCharW, charW=charW,
               bicharW=bicharW, leftW=leftW, leftb=leftb, rightW=rightW,
               rightb=rightb, Wih_l=Wih_l, Whh_l=Whh_l, bih_l=bih_l,
               bhh_l=bhh_l, Wih_r=Wih_r, Whh_r=Whh_r, bih_r=bih_r, bhh_r=bhh_r)
    shared = _prep_weights(inp, t_steps)

    cI = np.asarray(charIndexes, np.int64).astype(np.int32)[:, :t_steps]
    bI = np.asarray(bicharIndexes, np.int64).astype(np.int32)[:, :t_steps]
    # left features use bichar index of t+1; pad position -> zero row
    bI_l = np.concatenate(
        [bI[:, 1:], np.full((B, 1), BICHAR_NUM, np.int32)], axis=1)

    if t_steps not in _NC_CACHE:
        _NC_CACHE[t_steps] = build_nc(t_steps)
    nc = _NC_CACHE[t_steps]

    in_maps = []
    for k in range(NCORES):
        rows = slice(BC * k, BC * (k + 1))
        m = dict(shared)
        m["cidx"] = _idx_calls(cI[rows], t_steps)
        m["bidx_l"] = _idx_calls(bI_l[rows], t_steps)
        m["bidx_r"] = _idx_calls(bI[rows], t_steps)
        in_maps.append(m)

    res = run_bass_kernel_spmd(nc, in_maps, core_ids=list(range(NCORES)),
                               trace=_trace)
    if _trace:
        kernel._last_trace = res

    output = np.empty((B, t_steps, 2 * R), np.float32)
    for k in range(NCORES):
        r = res.results[k]
        # out_l/out_r are [t, b, 400]
        output[BC * k:BC * (k + 1), :, 0:R] = r["out_l"].transpose(1, 0, 2)
        output[BC * k:BC * (k + 1), :, R:2 * R] = r["out_r"].transpose(1, 0, 2)
    return output, np.asarray(hidden)


def run_profiled(inp_np):
    """Re-run with NTFF tracing; returns BassKernelResults (or None)."""
    kernel._last_trace = None
    kernel(_trace=True, **inp_np)
    return kernel._last_trace


def _build_in_maps(inp_np, t_steps=T):
    shared = _prep_weights(inp_np, t_steps)
    cI = np.asarray(inp_np["charIndexes"], np.int64).astype(np.int32)[:, :t_steps]
    bI = np.asarray(inp_np["bicharIndexes"], np.int64).astype(np.int32)[:, :t_steps]
    bI_l = np.concatenate(
        [bI[:, 1:], np.full((B, 1), BICHAR_NUM, np.int32)], axis=1)
    in_maps = []
    for k in range(NCORES):
        rows = slice(BC * k, BC * (k + 1))
        m = dict(shared)
        m["cidx"] = _idx_calls(cI[rows], t_steps)
        m["bidx_l"] = _idx_calls(bI_l[rows], t_steps)
        m["bidx_r"] = _idx_calls(bI[rows], t_steps)
        in_maps.append(m)
    return in_maps


def timed_hw_runs(inp_np, n_runs=4, t_steps=None):
    """Measure on-device execution time: jit once, pre-stage inputs on the
    devices, then time repeated executes (block_until_ready). Returns list of
    per-run seconds."""
    import time
    import jax
    from jax.sharding import Mesh, PartitionSpec, NamedSharding
    from jax.experimental.shard_map import shard_map
    from concourse import bass2jax

    t_steps = t_steps or T
    if t_steps not in _NC_CACHE:
        _NC_CACHE[t_steps] = build_nc(t_steps)
    nc = _NC_CACHE[t_steps]
    in_maps = _build_in_maps(inp_np, t_steps)

    bass2jax.install_neuronx_cc_hook()
    partition_name = (nc.partition_id_tensor.name
                      if nc.partition_id_tensor else None)
    in_names, out_names, out_avals, zero_outs = [], [], [], []
    for alloc in nc.m.functions[0].allocations:
        if not isinstance(alloc, mybir.MemoryLocationSet):
            continue
        name = alloc.memorylocations[0].name
        if alloc.kind == "ExternalInput":
            if name != partition_name:
                in_names.append(name)
        elif alloc.kind == "ExternalOutput":
            out_names.append(name)
            shape = tuple(alloc.tensor_shape)
            dtype = mybir.dt.np(alloc.dtype)
            out_avals.append(jax.core.ShapedArray(shape, dtype))
            zero_outs.append(np.zeros(shape, dtype))
    n_params = len(in_names)
    all_in_names = in_names + out_names
    if partition_name is not None:
        all_in_names = all_in_names + [partition_name]

    def _body(*args):
        operands = list(args)
        if partition_name is not None:
            operands.append(bass2jax.partition_id_tensor())
        outs = bass2jax._bass_exec_p.bind(
            *operands, out_avals=tuple(out_avals),
            in_names=tuple(all_in_names), out_names=tuple(out_names),
            lowering_input_output_aliases=(), sim_require_finite=True,
            sim_require_nnan=True, nc=nc)
        return tuple(outs)

    devices = jax.devices()[:NCORES]
    mesh = Mesh(np.asarray(devices), ("core",))
    spec = PartitionSpec("core")
    in_specs = (spec,) * (n_params + len(out_names))
    out_specs = (spec,) * len(out_names)
    fn = jax.jit(shard_map(_body, mesh=mesh, in_specs=in_specs,
                           out_specs=out_specs, check_rep=False),
                 keep_unused=True)

    sh = NamedSharding(mesh, spec)
    dev_in = [jax.device_put(
        np.concatenate([np.asarray(in_maps[c][n]) for c in range(NCORES)],
                       axis=0), sh) for n in in_names]
    dev_zero = [jax.device_put(
        np.zeros((NCORES * z.shape[0], *z.shape[1:]), z.dtype), sh)
        for z in zero_outs]

    jax.block_until_ready(fn(*dev_in, *dev_zero))  # warmup / compile
    times = []
    for _ in range(n_runs):
        t0 = time.perf_counter()
        jax.block_until_ready(fn(*dev_in, *dev_zero))
        times.append(time.perf_counter() - t0)
    return times


def timeline_estimate(t_steps=None):
    """Cost-model (TimelineSim) estimate of single-core exec time in ns."""
    from concourse.timeline_sim import TimelineSim
    t_steps = t_steps or T
    if t_steps not in _NC_CACHE:
        _NC_CACHE[t_steps] = build_nc(t_steps)
    return TimelineSim(_NC_CACHE[t_steps], trace=False).simulate()
